# revision 16
# baseline (speedup 1.0000x reference)
"""LIF spiking-neuron recurrence kernel for Trainium2 (8 NeuronCores, SPMD).

Problem: x [32, 100, 8192] f32, decay [1] f32.
    d = sigmoid(decay)
    mem_0 = x[:,0];  mem_t = mem_{t-1} * d * (1 - spike_{t-1}) + x[:,t]
    spike_t = (mem_t > 0.5);  out[:,t] = spike_t  (f32 0/1)

Device formulation (bit-exact vs the reference):
    W_{-1} = 0
    M_t = (W_{t-1} * d) + x_t
    W_t = (M_t <= 0.5) * M_t
spike_t = (M_t > 0.5) = (W_t == 0) exactly. The recurrence runs as ONE
custom DVE op per chunk over a persistent W buffer (out AP trails in0 by
F elements; the written state is read back ~250 cycles later).

Spike extraction/output — engine-balanced around the measured limits
(DVE is the scarce engine: the serial LIF chain alone costs ~27 us; the
HBM load floor is ~37 us; ACT runs 1 elem/cyc/lane with 2 passes needed
for a compare; GPSIMD tensor ops are ~18 cyc/elem and lock the shared
SBUF port, so it is not used):
  - ACT chunk-shares (early steps of each chunk): q = Square(1e19*W)
    into PSUM, s = Relu(1 - q) -> u8 {0,1}; stored as plain bytes.
  - DVE share (rest): custom SPIKE_PAIR op, one 1x pass fusing
    extraction and 2-bit packing: byte = (W_even==0) + 2*(W_odd==0),
    storing HALF the bytes for the same DVE cost as a plain is_equal.
Two output tensors (u8 spikes / 2-bit pairs); host reassembles.

Loads go on the sync HWDGE ring (nothing else queues there), stores on
the scalar ring interleaved with ACT compute; DVE has NO cross-engine
input dependencies (pair ops read W which DVE itself wrote), so the
serial LIF chain is never blocked.

Sharding: d-shard: core c owns d in [1024c, 1024c+1024); per-core
layout [128, T*256] with partition p = b*4 + (d_local//256). No
cross-core communication.
"""

from contextlib import ExitStack

import numpy as np

N_CORES = 8
B, T, D = 32, 100, 8192
P = 128          # SBUF partitions
F = 256          # free elements per timestep per core
THRESH = 0.5

# Load chunks (timesteps) and the per-chunk ACT share (first act_c steps
# extracted by ACT as u8; the rest go through the DVE pair op, so the
# non-ACT count must be even). Tail chunks are small and all-DVE so the
# post-load-floor serial tail stays short.
SCHED = [4, 12, 20, 20, 20, 16, 4, 4]
ACT_STEPS = [4, 12, 12, 12, 12, 12, 0, 0]

_BUILD_CACHE: dict = {}
_LIF_OP = None
_PAIR_OP = None


def _get_custom_ops():
    """Register the fused LIF-step and spike-pair DVE ops (idempotent)."""
    global _LIF_OP, _PAIR_OP
    if _LIF_OP is not None:
        return _LIF_OP, _PAIR_OP
    from concourse.dve_ops import (
        CUSTOM_DVE_SPECS, OPS, _SUB_OPCODE_FOR_NAME, DveOp,
    )
    from concourse.dve_spec import C0, C1, Spec, Src0, Src1, eq, lower
    from concourse.dve_table_gen import dve_ver_for
    from concourse.dve_uop import DveOpSpec

    assert dve_ver_for("TRN2") == "v3"

    def register(name, spec):
        if name in _SUB_OPCODE_FOR_NAME:
            return next(op for op in OPS if op.name == name)
        row = max(_SUB_OPCODE_FOR_NAME.values()) + 1
        assert row < 0x20
        _SUB_OPCODE_FOR_NAME[name] = row
        tmp = DveOpSpec(name=name, opcode=row, uops=lower(spec, ver="v3"),
                        rd1_en=True)
        op = DveOp(name, spec, subdim=False, uops_sha={"v3": tmp.sha("v3")})
        OPS.append(op)
        CUSTOM_DVE_SPECS[name] = spec
        return op

    M = Src0 * C0 + Src1

    def _lif_ref(in0, in1, s0, s1, imm2):
        m = (in0.astype(np.float32) * np.float32(s0)
             + in1.astype(np.float32)).astype(np.float32)
        return np.where(m <= np.float32(s1), m,
                        np.float32(0.0)).astype(np.float32)

    _LIF_OP = register("LIF_STEP_ANT",
                       Spec(body=M * (M <= C1), reference=_lif_ref))

    def _pair_ref(in0, in1, s0, s1, imm2):
        return ((in0 == np.float32(s0)).astype(np.float32)
                + (in1 == np.float32(s0)).astype(np.float32)
                * np.float32(s1)).astype(np.float32)

    _PAIR_OP = register("SPIKE_PAIR_ANT",
                        Spec(body=eq(Src0, C0) + eq(Src1, C0) * C1,
                             reference=_pair_ref))
    return _LIF_OP, _PAIR_OP


def _splits():
    """Per-chunk (t0, tc, act_c, pair_c, u8_off, p2_off) in elements."""
    out = []
    t0 = u8o = p2o = 0
    for tc, ac in zip(SCHED, ACT_STEPS):
        pc = tc - ac
        assert pc % 2 == 0
        out.append((t0, tc, ac, pc, u8o, p2o))
        t0 += tc
        u8o += ac * F
        p2o += (pc // 2) * F
    return out, u8o, p2o


def _build_nc(t_steps: int, d_imm: float):
    import concourse.tile as tile
    from concourse import bacc, bass, mybir

    assert t_steps == T, "schedule is hardcoded for T=100"
    lif_op, pair_op = _get_custom_ops()
    assert sum(SCHED) == T
    chunks, u8_total, p2_total = _splits()

    AF = mybir.ActivationFunctionType

    nc = bacc.Bacc("TRN2", debug=False, target_bir_lowering=False)
    x_in = nc.dram_tensor("x", [P, T * F], mybir.dt.float32,
                          kind="ExternalInput")
    su8_out = nc.dram_tensor("su8", [P, u8_total], mybir.dt.uint8,
                             kind="ExternalOutput")
    sp2_out = nc.dram_tensor("sp2", [P, p2_total], mybir.dt.uint8,
                             kind="ExternalOutput")

    max_tc = max(SCHED)
    max_ac = max(ACT_STEPS)
    max_pc = max(tc - ac for tc, ac in zip(SCHED, ACT_STEPS))

    with tile.TileContext(nc) as tcx, ExitStack() as ctx:
        xpool = ctx.enter_context(tcx.tile_pool(name="xp", bufs=3))
        opool = ctx.enter_context(tcx.tile_pool(name="op", bufs=2))
        qpool = ctx.enter_context(
            tcx.tile_pool(name="qp", bufs=1, space=bass.MemorySpace.PSUM))
        spool = ctx.enter_context(tcx.tile_pool(name="sp", bufs=1))

        # Persistent state: W[:, t*F:(t+1)*F] holds W_{t-1} (slot 0 = 0).
        wbuf = spool.tile([P, (T + 1) * F], mybir.dt.float32)
        scr = spool.tile([P, 32], mybir.dt.float32)

        nc.vector.memset(wbuf[:, 0:F], 0.0)
        nc.scalar.memzero(scr[:, :])
        nc.scalar.activation(out=scr[:, :], in_=scr[:, :], func=AF.Square)

        def emit_act_extract(t0, ac, u8o):
            n = ac * F
            qt = qpool.tile([P, max_ac * F], mybir.dt.float32, tag="qt")
            st = opool.tile([P, max_ac * F], mybir.dt.uint8, tag="su")
            nc.scalar.activation(out=qt[:, :n],
                                 in_=wbuf[:, (t0 + 1) * F:(t0 + ac + 1) * F],
                                 func=AF.Square, scale=1e19)
            nc.scalar.activation(out=st[:, :n], in_=qt[:, :n], func=AF.Relu,
                                 bias=1.0, scale=-1.0)
            nc.sync.dma_start(out=su8_out[:, u8o:u8o + n], in_=st[:, :n])

        def emit_pair(t0, ac, pc, p2o):
            n = (pc // 2) * F
            pt = opool.tile([P, (max_pc // 2) * F], mybir.dt.uint8, tag="sp")
            wv = wbuf[:, (t0 + ac + 1) * F:(t0 + ac + pc + 1) * F]
            wv = wv.rearrange("p (g two f) -> p g two f", g=pc // 2, two=2)
            ov = pt[:, :n].rearrange("p (g f) -> p g f", g=pc // 2)
            nc.vector._custom_dve(pair_op, out=ov, in0=wv[:, :, 0, :],
                                  in1=wv[:, :, 1, :], s0=0.0, s1=2.0)
            nc.sync.dma_start(out=sp2_out[:, p2o:p2o + n], in_=pt[:, :n])

        prev = None
        for ci, (t0, tc, ac, pc, u8o, p2o) in enumerate(chunks):
            xt = xpool.tile([P, max_tc * F], mybir.dt.float32, tag="xt")
            nc.sync.dma_start(out=xt[:, :tc * F],
                              in_=x_in[:, t0 * F:(t0 + tc) * F])
            nc.vector._custom_dve(
                lif_op,
                out=wbuf[:, (t0 + 1) * F:(t0 + tc + 1) * F],
                in0=wbuf[:, t0 * F:(t0 + tc) * F],
                in1=xt[:, :tc * F],
                s0=d_imm, s1=THRESH)
            if prev is not None:
                pt0, _, pac, ppc, pu8o, pp2o = prev
                if pac:
                    emit_act_extract(pt0, pac, pu8o)
                if ppc:
                    emit_pair(pt0, pac, ppc, pp2o)
            prev = chunks[ci]
        pt0, _, pac, ppc, pu8o, pp2o = prev
        if pac:
            emit_act_extract(pt0, pac, pu8o)
        if ppc:
            emit_pair(pt0, pac, ppc, pp2o)
    nc.compile()
    return nc


def _get_nc(t_steps: int, d_imm: float):
    key = (t_steps, np.float32(d_imm).tobytes())
    if key not in _BUILD_CACHE:
        _BUILD_CACHE[key] = _build_nc(t_steps, d_imm)
    return _BUILD_CACHE[key]


def _shard_x(x: np.ndarray) -> list[np.ndarray]:
    b, t, d = x.shape
    # [b, t, core, chunk, 256] -> [core, b, chunk, t, 256] -> [core, 128, t*256]
    xr = x.reshape(b, t, N_CORES, 4, F).transpose(2, 0, 3, 1, 4)
    xr = np.ascontiguousarray(xr).reshape(N_CORES, P, t * F)
    return [xr[c] for c in range(N_CORES)]


def _unshard_spikes(su8: np.ndarray, sp2: np.ndarray, t: int) -> np.ndarray:
    # Reassemble per-step spike bytes [core, 128, T, F] from the two
    # output formats, then unshard to [B, T, D].
    chunks, _, _ = _splits()
    sp = np.empty((N_CORES, P, t, F), dtype=np.uint8)
    for t0, tc, ac, pc, u8o, p2o in chunks:
        if ac:
            sp[:, :, t0:t0 + ac, :] = su8[:, :, u8o:u8o + ac * F].reshape(
                N_CORES, P, ac, F)
        if pc:
            pb = sp2[:, :, p2o:p2o + (pc // 2) * F].reshape(
                N_CORES, P, pc // 2, F)
            sp[:, :, t0 + ac:t0 + tc:2, :] = pb & 1
            sp[:, :, t0 + ac + 1:t0 + tc:2, :] = pb >> 1
    sr = sp.astype(np.float32).reshape(N_CORES, B, 4, t, F)
    sr = sr.transpose(1, 3, 0, 2, 4)
    return np.ascontiguousarray(sr).reshape(B, t, N_CORES * 4 * F)


def _sigmoid_f32(decay: np.ndarray) -> np.float32:
    import jax
    import jax.numpy as jnp
    d = np.asarray(jax.nn.sigmoid(jnp.asarray(decay, jnp.float32)))
    return np.float32(d.reshape(-1)[0])


def kernel(x: np.ndarray, decay: np.ndarray) -> np.ndarray:
    from concourse.bass_utils import run_bass_kernel_spmd

    x = np.asarray(x, dtype=np.float32)
    b, t, d = x.shape
    d_f32 = _sigmoid_f32(np.asarray(decay))

    nc = _get_nc(t, float(d_f32))
    shards = _shard_x(x)
    in_maps = [{"x": np.ascontiguousarray(s)} for s in shards]
    res = run_bass_kernel_spmd(nc, in_maps, core_ids=list(range(N_CORES)))
    su8 = np.stack([np.asarray(res.results[c]["su8"])
                    for c in range(N_CORES)], axis=0)
    sp2 = np.stack([np.asarray(res.results[c]["sp2"])
                    for c in range(N_CORES)], axis=0)
    return _unshard_spikes(su8, sp2, t)


# revision 18
# speedup vs baseline: 1.0450x; 1.0450x over previous
"""LIF spiking-neuron recurrence kernel for Trainium2 (8 NeuronCores, SPMD).

Problem: x [32, 100, 8192] f32, decay [1] f32.
    d = sigmoid(decay)
    mem_0 = x[:,0];  mem_t = mem_{t-1} * d * (1 - spike_{t-1}) + x[:,t]
    spike_t = (mem_t > 0.5);  out[:,t] = spike_t  (f32 0/1)

Device formulation (bit-exact vs the reference):
    W_{-1} = 0
    M_t = (W_{t-1} * d) + x_t
    W_t = (M_t <= 0.5) * M_t
spike_t = (M_t > 0.5) = (W_t == 0) exactly. The recurrence runs as ONE
custom DVE op per chunk over a persistent W buffer (out AP trails in0 by
F elements; the written state is read back ~250 cycles later).

Spike extraction/output — engine-balanced around the measured limits
(DVE is the scarce engine: the serial LIF chain alone costs ~27 us; the
HBM load floor is ~37 us; ACT runs 1 elem/cyc/lane with 2 passes needed
for a compare; GPSIMD tensor ops are ~18 cyc/elem and lock the shared
SBUF port, so it is not used):
  - ACT chunk-shares (early steps of each chunk): q = Square(1e19*W)
    into PSUM, s = Relu(1 - q) -> u8 {0,1}; stored as plain bytes.
  - DVE share (rest): custom SPIKE_PAIR op, one 1x pass fusing
    extraction and 2-bit packing: byte = (W_even==0) + 2*(W_odd==0),
    storing HALF the bytes for the same DVE cost as a plain is_equal.
Two output tensors (u8 spikes / 2-bit pairs); host reassembles.

Loads go on the sync HWDGE ring (nothing else queues there), stores on
the scalar ring interleaved with ACT compute; DVE has NO cross-engine
input dependencies (pair ops read W which DVE itself wrote), so the
serial LIF chain is never blocked.

Sharding: d-shard: core c owns d in [1024c, 1024c+1024); per-core
layout [128, T*256] with partition p = b*4 + (d_local//256). No
cross-core communication.
"""

from contextlib import ExitStack

import numpy as np

N_CORES = 8
B, T, D = 32, 100, 8192
P = 128          # SBUF partitions
F = 256          # free elements per timestep per core
THRESH = 0.5

# Load chunks (timesteps) and the per-chunk ACT share (first act_c steps
# extracted by ACT as u8; the rest go through the DVE pair op, so the
# non-ACT count must be even). Tail chunks are small and all-DVE so the
# post-load-floor serial tail stays short.
SCHED = [4, 12, 20, 20, 20, 16, 4, 4]
ACT_STEPS = [4, 12, 12, 12, 12, 12, 0, 0]

_BUILD_CACHE: dict = {}
_LIF_OP = None
_PAIR_OP = None


def _get_custom_ops():
    """Register the fused LIF-step and spike-pair DVE ops (idempotent)."""
    global _LIF_OP, _PAIR_OP
    if _LIF_OP is not None:
        return _LIF_OP, _PAIR_OP
    from concourse.dve_ops import (
        CUSTOM_DVE_SPECS, OPS, _SUB_OPCODE_FOR_NAME, DveOp,
    )
    from concourse.dve_spec import C0, C1, Spec, Src0, Src1, eq, lower
    from concourse.dve_table_gen import dve_ver_for
    from concourse.dve_uop import DveOpSpec

    assert dve_ver_for("TRN2") == "v3"

    def register(name, spec):
        if name in _SUB_OPCODE_FOR_NAME:
            return next(op for op in OPS if op.name == name)
        row = max(_SUB_OPCODE_FOR_NAME.values()) + 1
        assert row < 0x20
        _SUB_OPCODE_FOR_NAME[name] = row
        tmp = DveOpSpec(name=name, opcode=row, uops=lower(spec, ver="v3"),
                        rd1_en=True)
        op = DveOp(name, spec, subdim=False, uops_sha={"v3": tmp.sha("v3")})
        OPS.append(op)
        CUSTOM_DVE_SPECS[name] = spec
        return op

    M = Src0 * C0 + Src1

    def _lif_ref(in0, in1, s0, s1, imm2):
        m = (in0.astype(np.float32) * np.float32(s0)
             + in1.astype(np.float32)).astype(np.float32)
        return np.where(m <= np.float32(s1), m,
                        np.float32(0.0)).astype(np.float32)

    _LIF_OP = register("LIF_STEP_ANT",
                       Spec(body=M * (M <= C1), reference=_lif_ref))

    def _pair_ref(in0, in1, s0, s1, imm2):
        return ((in0 == np.float32(s0)).astype(np.float32)
                + (in1 == np.float32(s0)).astype(np.float32)
                * np.float32(s1)).astype(np.float32)

    _PAIR_OP = register("SPIKE_PAIR_ANT",
                        Spec(body=eq(Src0, C0) + eq(Src1, C0) * C1,
                             reference=_pair_ref))
    return _LIF_OP, _PAIR_OP


def _splits():
    """Per-chunk (t0, tc, act_c, pair_c, u8_off, p2_off) in elements."""
    out = []
    t0 = u8o = p2o = 0
    for tc, ac in zip(SCHED, ACT_STEPS):
        pc = tc - ac
        assert pc % 2 == 0
        out.append((t0, tc, ac, pc, u8o, p2o))
        t0 += tc
        u8o += ac * F
        p2o += (pc // 2) * F
    return out, u8o, p2o


def _build_nc(t_steps: int, d_imm: float):
    import concourse.tile as tile
    from concourse import bacc, bass, mybir

    assert t_steps == T, "schedule is hardcoded for T=100"
    lif_op, pair_op = _get_custom_ops()
    assert sum(SCHED) == T
    chunks, u8_total, p2_total = _splits()

    AF = mybir.ActivationFunctionType

    nc = bacc.Bacc("TRN2", debug=False, target_bir_lowering=False)
    x_in = nc.dram_tensor("x", [P, T * F], mybir.dt.float32,
                          kind="ExternalInput")
    su8_out = nc.dram_tensor("su8", [P, u8_total], mybir.dt.uint8,
                             kind="ExternalOutput")
    sp2_out = nc.dram_tensor("sp2", [P, p2_total], mybir.dt.uint8,
                             kind="ExternalOutput")

    max_tc = max(SCHED)
    max_ac = max(ACT_STEPS)
    max_pc = max(tc - ac for tc, ac in zip(SCHED, ACT_STEPS))

    with tile.TileContext(nc) as tcx, ExitStack() as ctx:
        xpool = ctx.enter_context(tcx.tile_pool(name="xp", bufs=3))
        opool = ctx.enter_context(tcx.tile_pool(name="op", bufs=2))
        qpool = ctx.enter_context(
            tcx.tile_pool(name="qp", bufs=1, space=bass.MemorySpace.PSUM))
        spool = ctx.enter_context(tcx.tile_pool(name="sp", bufs=1))

        # Persistent state: W[:, t*F:(t+1)*F] holds W_{t-1} (slot 0 = 0).
        wbuf = spool.tile([P, (T + 1) * F], mybir.dt.float32)
        scr = spool.tile([P, 32], mybir.dt.float32)

        nc.vector.memset(wbuf[:, 0:F], 0.0)
        nc.scalar.memzero(scr[:, :])
        nc.scalar.activation(out=scr[:, :], in_=scr[:, :], func=AF.Square)

        pending_stores = []

        def emit_act_extract(t0, ac, u8o):
            n = ac * F
            qt = qpool.tile([P, max_ac * F], mybir.dt.float32, tag="qt")
            st = opool.tile([P, max_ac * F], mybir.dt.uint8, tag="su",
                            bufs=len([a for a in ACT_STEPS if a]))
            nc.scalar.activation(out=qt[:, :n],
                                 in_=wbuf[:, (t0 + 1) * F:(t0 + ac + 1) * F],
                                 func=AF.Square, scale=1e19)
            nc.scalar.activation(out=st[:, :n], in_=qt[:, :n], func=AF.Relu,
                                 bias=1.0, scale=-1.0)
            pending_stores.append((su8_out[:, u8o:u8o + n], st[:, :n]))

        def emit_pair(t0, ac, pc, p2o):
            n = (pc // 2) * F
            pt = opool.tile([P, (max_pc // 2) * F], mybir.dt.uint8, tag="sp",
                            bufs=len([1 for tc, a in zip(SCHED, ACT_STEPS) if tc - a]))
            wv = wbuf[:, (t0 + ac + 1) * F:(t0 + ac + pc + 1) * F]
            wv = wv.rearrange("p (g two f) -> p g two f", g=pc // 2, two=2)
            ov = pt[:, :n].rearrange("p (g f) -> p g f", g=pc // 2)
            nc.vector._custom_dve(pair_op, out=ov, in0=wv[:, :, 0, :],
                                  in1=wv[:, :, 1, :], s0=0.0, s1=2.0)
            pending_stores.append((sp2_out[:, p2o:p2o + n], pt[:, :n]))

        prev = None
        for ci, (t0, tc, ac, pc, u8o, p2o) in enumerate(chunks):
            xt = xpool.tile([P, max_tc * F], mybir.dt.float32, tag="xt")
            nc.sync.dma_start(out=xt[:, :tc * F],
                              in_=x_in[:, t0 * F:(t0 + tc) * F])
            nc.vector._custom_dve(
                lif_op,
                out=wbuf[:, (t0 + 1) * F:(t0 + tc + 1) * F],
                in0=wbuf[:, t0 * F:(t0 + tc) * F],
                in1=xt[:, :tc * F],
                s0=d_imm, s1=THRESH)
            if prev is not None:
                pt0, _, pac, ppc, pu8o, pp2o = prev
                if pac:
                    emit_act_extract(pt0, pac, pu8o)
                if ppc:
                    emit_pair(pt0, pac, ppc, pp2o)
            prev = chunks[ci]
        pt0, _, pac, ppc, pu8o, pp2o = prev
        if pac:
            emit_act_extract(pt0, pac, pu8o)
        if ppc:
            emit_pair(pt0, pac, ppc, pp2o)
        for out_ap, in_ap in pending_stores:
            nc.sync.dma_start(out=out_ap, in_=in_ap)
    nc.compile()
    return nc


def _get_nc(t_steps: int, d_imm: float):
    key = (t_steps, np.float32(d_imm).tobytes())
    if key not in _BUILD_CACHE:
        _BUILD_CACHE[key] = _build_nc(t_steps, d_imm)
    return _BUILD_CACHE[key]


def _shard_x(x: np.ndarray) -> list[np.ndarray]:
    b, t, d = x.shape
    # [b, t, core, chunk, 256] -> [core, b, chunk, t, 256] -> [core, 128, t*256]
    xr = x.reshape(b, t, N_CORES, 4, F).transpose(2, 0, 3, 1, 4)
    xr = np.ascontiguousarray(xr).reshape(N_CORES, P, t * F)
    return [xr[c] for c in range(N_CORES)]


def _unshard_spikes(su8: np.ndarray, sp2: np.ndarray, t: int) -> np.ndarray:
    # Reassemble per-step spike bytes [core, 128, T, F] from the two
    # output formats, then unshard to [B, T, D].
    chunks, _, _ = _splits()
    sp = np.empty((N_CORES, P, t, F), dtype=np.uint8)
    for t0, tc, ac, pc, u8o, p2o in chunks:
        if ac:
            sp[:, :, t0:t0 + ac, :] = su8[:, :, u8o:u8o + ac * F].reshape(
                N_CORES, P, ac, F)
        if pc:
            pb = sp2[:, :, p2o:p2o + (pc // 2) * F].reshape(
                N_CORES, P, pc // 2, F)
            sp[:, :, t0 + ac:t0 + tc:2, :] = pb & 1
            sp[:, :, t0 + ac + 1:t0 + tc:2, :] = pb >> 1
    sr = sp.astype(np.float32).reshape(N_CORES, B, 4, t, F)
    sr = sr.transpose(1, 3, 0, 2, 4)
    return np.ascontiguousarray(sr).reshape(B, t, N_CORES * 4 * F)


def _sigmoid_f32(decay: np.ndarray) -> np.float32:
    import jax
    import jax.numpy as jnp
    d = np.asarray(jax.nn.sigmoid(jnp.asarray(decay, jnp.float32)))
    return np.float32(d.reshape(-1)[0])


def kernel(x: np.ndarray, decay: np.ndarray) -> np.ndarray:
    from concourse.bass_utils import run_bass_kernel_spmd

    x = np.asarray(x, dtype=np.float32)
    b, t, d = x.shape
    d_f32 = _sigmoid_f32(np.asarray(decay))

    nc = _get_nc(t, float(d_f32))
    shards = _shard_x(x)
    in_maps = [{"x": np.ascontiguousarray(s)} for s in shards]
    res = run_bass_kernel_spmd(nc, in_maps, core_ids=list(range(N_CORES)))
    su8 = np.stack([np.asarray(res.results[c]["su8"])
                    for c in range(N_CORES)], axis=0)
    sp2 = np.stack([np.asarray(res.results[c]["sp2"])
                    for c in range(N_CORES)], axis=0)
    return _unshard_spikes(su8, sp2, t)


# revision 19
# speedup vs baseline: 1.0765x; 1.0301x over previous
"""LIF spiking-neuron recurrence kernel for Trainium2 (8 NeuronCores, SPMD).

Problem: x [32, 100, 8192] f32, decay [1] f32.
    d = sigmoid(decay)
    mem_0 = x[:,0];  mem_t = mem_{t-1} * d * (1 - spike_{t-1}) + x[:,t]
    spike_t = (mem_t > 0.5);  out[:,t] = spike_t  (f32 0/1)

Device formulation (bit-exact vs the reference):
    W_{-1} = 0
    M_t = (W_{t-1} * d) + x_t
    W_t = (M_t <= 0.5) * M_t
spike_t = (M_t > 0.5) = (W_t == 0) exactly. The recurrence runs as ONE
custom DVE op per chunk over a persistent W buffer (out AP trails in0 by
F elements; the written state is read back ~250 cycles later).

Spike extraction/output — engine-balanced around the measured limits
(DVE is the scarce engine: the serial LIF chain alone costs ~27 us; the
HBM load floor is ~37 us; ACT runs 1 elem/cyc/lane with 2 passes needed
for a compare; GPSIMD tensor ops are ~18 cyc/elem and lock the shared
SBUF port, so it is not used):
  - ACT chunk-shares (early steps of each chunk): q = Square(1e19*W)
    into PSUM, s = Relu(1 - q) -> u8 {0,1}; stored as plain bytes.
  - DVE share (rest): custom SPIKE_PAIR op, one 1x pass fusing
    extraction and 2-bit packing: byte = (W_even==0) + 2*(W_odd==0),
    storing HALF the bytes for the same DVE cost as a plain is_equal.
Two output tensors (u8 spikes / 2-bit pairs); host reassembles.

Loads go on the sync HWDGE ring (nothing else queues there), stores on
the scalar ring interleaved with ACT compute; DVE has NO cross-engine
input dependencies (pair ops read W which DVE itself wrote), so the
serial LIF chain is never blocked.

Sharding: d-shard: core c owns d in [1024c, 1024c+1024); per-core
layout [128, T*256] with partition p = b*4 + (d_local//256). No
cross-core communication.
"""

from contextlib import ExitStack

import numpy as np

N_CORES = 8
B, T, D = 32, 100, 8192
P = 128          # SBUF partitions
F = 256          # free elements per timestep per core
THRESH = 0.5

# Load chunks (timesteps) and the per-chunk ACT share (first act_c steps
# extracted by ACT as u8; the rest go through the DVE pair op, so the
# non-ACT count must be even). Tail chunks are small and all-DVE so the
# post-load-floor serial tail stays short.
SCHED = [4, 12, 20, 20, 20, 16, 4, 4]
ACT_STEPS = [4, 8, 8, 8, 8, 8, 0, 0]

_BUILD_CACHE: dict = {}
_LIF_OP = None
_PAIR_OP = None


def _get_custom_ops():
    """Register the fused LIF-step and spike-pair DVE ops (idempotent)."""
    global _LIF_OP, _PAIR_OP
    if _LIF_OP is not None:
        return _LIF_OP, _PAIR_OP
    from concourse.dve_ops import (
        CUSTOM_DVE_SPECS, OPS, _SUB_OPCODE_FOR_NAME, DveOp,
    )
    from concourse.dve_spec import C0, C1, Spec, Src0, Src1, eq, lower
    from concourse.dve_table_gen import dve_ver_for
    from concourse.dve_uop import DveOpSpec

    assert dve_ver_for("TRN2") == "v3"

    def register(name, spec):
        if name in _SUB_OPCODE_FOR_NAME:
            return next(op for op in OPS if op.name == name)
        row = max(_SUB_OPCODE_FOR_NAME.values()) + 1
        assert row < 0x20
        _SUB_OPCODE_FOR_NAME[name] = row
        tmp = DveOpSpec(name=name, opcode=row, uops=lower(spec, ver="v3"),
                        rd1_en=True)
        op = DveOp(name, spec, subdim=False, uops_sha={"v3": tmp.sha("v3")})
        OPS.append(op)
        CUSTOM_DVE_SPECS[name] = spec
        return op

    M = Src0 * C0 + Src1

    def _lif_ref(in0, in1, s0, s1, imm2):
        m = (in0.astype(np.float32) * np.float32(s0)
             + in1.astype(np.float32)).astype(np.float32)
        return np.where(m <= np.float32(s1), m,
                        np.float32(0.0)).astype(np.float32)

    _LIF_OP = register("LIF_STEP_ANT",
                       Spec(body=M * (M <= C1), reference=_lif_ref))

    def _pair_ref(in0, in1, s0, s1, imm2):
        return ((in0 == np.float32(s0)).astype(np.float32)
                + (in1 == np.float32(s0)).astype(np.float32)
                * np.float32(s1)).astype(np.float32)

    _PAIR_OP = register("SPIKE_PAIR_ANT",
                        Spec(body=eq(Src0, C0) + eq(Src1, C0) * C1,
                             reference=_pair_ref))
    return _LIF_OP, _PAIR_OP


def _splits():
    """Per-chunk (t0, tc, act_c, pair_c, u8_off, p2_off) in elements."""
    out = []
    t0 = u8o = p2o = 0
    for tc, ac in zip(SCHED, ACT_STEPS):
        pc = tc - ac
        assert pc % 2 == 0
        out.append((t0, tc, ac, pc, u8o, p2o))
        t0 += tc
        u8o += ac * F
        p2o += (pc // 2) * F
    return out, u8o, p2o


def _build_nc(t_steps: int, d_imm: float):
    import concourse.tile as tile
    from concourse import bacc, bass, mybir

    assert t_steps == T, "schedule is hardcoded for T=100"
    lif_op, pair_op = _get_custom_ops()
    assert sum(SCHED) == T
    chunks, u8_total, p2_total = _splits()

    AF = mybir.ActivationFunctionType

    nc = bacc.Bacc("TRN2", debug=False, target_bir_lowering=False)
    x_in = nc.dram_tensor("x", [P, T * F], mybir.dt.float32,
                          kind="ExternalInput")
    su8_out = nc.dram_tensor("su8", [P, u8_total], mybir.dt.uint8,
                             kind="ExternalOutput")
    sp2_out = nc.dram_tensor("sp2", [P, p2_total], mybir.dt.uint8,
                             kind="ExternalOutput")

    max_tc = max(SCHED)
    max_ac = max(ACT_STEPS)
    max_pc = max(tc - ac for tc, ac in zip(SCHED, ACT_STEPS))

    with tile.TileContext(nc) as tcx, ExitStack() as ctx:
        xpool = ctx.enter_context(tcx.tile_pool(name="xp", bufs=3))
        opool = ctx.enter_context(tcx.tile_pool(name="op", bufs=2))
        qpool = ctx.enter_context(
            tcx.tile_pool(name="qp", bufs=1, space=bass.MemorySpace.PSUM))
        spool = ctx.enter_context(tcx.tile_pool(name="sp", bufs=1))

        # Persistent state: W[:, t*F:(t+1)*F] holds W_{t-1} (slot 0 = 0).
        wbuf = spool.tile([P, (T + 1) * F], mybir.dt.float32)
        scr = spool.tile([P, 32], mybir.dt.float32)

        nc.vector.memset(wbuf[:, 0:F], 0.0)
        nc.scalar.memzero(scr[:, :])
        nc.scalar.activation(out=scr[:, :], in_=scr[:, :], func=AF.Square)

        pending_stores = []

        def emit_act_extract(t0, ac, u8o):
            n = ac * F
            qt = qpool.tile([P, max_ac * F], mybir.dt.float32, tag="qt")
            st = opool.tile([P, max_ac * F], mybir.dt.uint8, tag="su",
                            bufs=len([a for a in ACT_STEPS if a]))
            nc.scalar.activation(out=qt[:, :n],
                                 in_=wbuf[:, (t0 + 1) * F:(t0 + ac + 1) * F],
                                 func=AF.Square, scale=1e19)
            nc.scalar.activation(out=st[:, :n], in_=qt[:, :n], func=AF.Relu,
                                 bias=1.0, scale=-1.0)
            nc.scalar.dma_start(out=su8_out[:, u8o:u8o + n], in_=st[:, :n])

        def emit_pair(t0, ac, pc, p2o):
            n = (pc // 2) * F
            pt = opool.tile([P, (max_pc // 2) * F], mybir.dt.uint8, tag="sp",
                            bufs=len([1 for tc, a in zip(SCHED, ACT_STEPS) if tc - a]))
            wv = wbuf[:, (t0 + ac + 1) * F:(t0 + ac + pc + 1) * F]
            wv = wv.rearrange("p (g two f) -> p g two f", g=pc // 2, two=2)
            ov = pt[:, :n].rearrange("p (g f) -> p g f", g=pc // 2)
            nc.vector._custom_dve(pair_op, out=ov, in0=wv[:, :, 0, :],
                                  in1=wv[:, :, 1, :], s0=0.0, s1=2.0)
            nc.scalar.dma_start(out=sp2_out[:, p2o:p2o + n], in_=pt[:, :n])

        prev = None
        for ci, (t0, tc, ac, pc, u8o, p2o) in enumerate(chunks):
            xt = xpool.tile([P, max_tc * F], mybir.dt.float32, tag="xt")
            nc.sync.dma_start(out=xt[:, :tc * F],
                              in_=x_in[:, t0 * F:(t0 + tc) * F])
            nc.vector._custom_dve(
                lif_op,
                out=wbuf[:, (t0 + 1) * F:(t0 + tc + 1) * F],
                in0=wbuf[:, t0 * F:(t0 + tc) * F],
                in1=xt[:, :tc * F],
                s0=d_imm, s1=THRESH)
            if prev is not None:
                pt0, _, pac, ppc, pu8o, pp2o = prev
                if pac:
                    emit_act_extract(pt0, pac, pu8o)
                if ppc:
                    emit_pair(pt0, pac, ppc, pp2o)
            prev = chunks[ci]
        pt0, _, pac, ppc, pu8o, pp2o = prev
        if pac:
            emit_act_extract(pt0, pac, pu8o)
        if ppc:
            emit_pair(pt0, pac, ppc, pp2o)
    nc.compile()
    return nc


def _get_nc(t_steps: int, d_imm: float):
    key = (t_steps, np.float32(d_imm).tobytes())
    if key not in _BUILD_CACHE:
        _BUILD_CACHE[key] = _build_nc(t_steps, d_imm)
    return _BUILD_CACHE[key]


def _shard_x(x: np.ndarray) -> list[np.ndarray]:
    b, t, d = x.shape
    # [b, t, core, chunk, 256] -> [core, b, chunk, t, 256] -> [core, 128, t*256]
    xr = x.reshape(b, t, N_CORES, 4, F).transpose(2, 0, 3, 1, 4)
    xr = np.ascontiguousarray(xr).reshape(N_CORES, P, t * F)
    return [xr[c] for c in range(N_CORES)]


def _unshard_spikes(su8: np.ndarray, sp2: np.ndarray, t: int) -> np.ndarray:
    # Reassemble per-step spike bytes [core, 128, T, F] from the two
    # output formats, then unshard to [B, T, D].
    chunks, _, _ = _splits()
    sp = np.empty((N_CORES, P, t, F), dtype=np.uint8)
    for t0, tc, ac, pc, u8o, p2o in chunks:
        if ac:
            sp[:, :, t0:t0 + ac, :] = su8[:, :, u8o:u8o + ac * F].reshape(
                N_CORES, P, ac, F)
        if pc:
            pb = sp2[:, :, p2o:p2o + (pc // 2) * F].reshape(
                N_CORES, P, pc // 2, F)
            sp[:, :, t0 + ac:t0 + tc:2, :] = pb & 1
            sp[:, :, t0 + ac + 1:t0 + tc:2, :] = pb >> 1
    sr = sp.astype(np.float32).reshape(N_CORES, B, 4, t, F)
    sr = sr.transpose(1, 3, 0, 2, 4)
    return np.ascontiguousarray(sr).reshape(B, t, N_CORES * 4 * F)


def _sigmoid_f32(decay: np.ndarray) -> np.float32:
    import jax
    import jax.numpy as jnp
    d = np.asarray(jax.nn.sigmoid(jnp.asarray(decay, jnp.float32)))
    return np.float32(d.reshape(-1)[0])


def kernel(x: np.ndarray, decay: np.ndarray) -> np.ndarray:
    from concourse.bass_utils import run_bass_kernel_spmd

    x = np.asarray(x, dtype=np.float32)
    b, t, d = x.shape
    d_f32 = _sigmoid_f32(np.asarray(decay))

    nc = _get_nc(t, float(d_f32))
    shards = _shard_x(x)
    in_maps = [{"x": np.ascontiguousarray(s)} for s in shards]
    res = run_bass_kernel_spmd(nc, in_maps, core_ids=list(range(N_CORES)))
    su8 = np.stack([np.asarray(res.results[c]["su8"])
                    for c in range(N_CORES)], axis=0)
    sp2 = np.stack([np.asarray(res.results[c]["sp2"])
                    for c in range(N_CORES)], axis=0)
    return _unshard_spikes(su8, sp2, t)
